# revision 11
# baseline (speedup 1.0000x reference)
"""GraphSAGE-mean 2-layer GNN kernel for 8 Trainium2 NeuronCores.

Strategy: dst-shard nodes across 8 cores (25000 each). Per layer, gathers
run as 8 owner-phases so the gather table is a 25001-row window (int16
indexable by dma_gather). Within each phase, dst nodes are ranked by
phase-degree (host index prep) so the padded per-chunk layout is tight;
a single dma_gather per ~96-column group fetches 128 rows/column from the
256B-stride table; DVE segmented-reduces produce the phase partial
[rank -> sum]; one dma_scatter_add per phase permutes+accumulates the
partial into a common natural-order DRAM accumulator (f32, exact).
Layer 2 reuses the identical index streams against a device-built
p = h @ W2_neigh table (AllGather + restride to 256B rows).
"""

import os
import sys

sys.path.insert(0, "/opt/trn_rl_repo")

import numpy as np

import concourse.bacc as bacc
import concourse.bass as bass
import concourse.tile as tile
from concourse import mybir
from concourse.bass_utils import run_bass_kernel_spmd
from concourse.masks import make_identity

F32 = mybir.dt.float32
BF16 = mybir.dt.bfloat16
I16 = mybir.dt.int16

NCORES = 8
P = 128
N_NODES = 200000
NPC = N_NODES // NCORES          # 25000
NCH = (NPC + P - 1) // P         # 196
NPAD = NCH * P                   # 25088
STEP = 64                        # table row stride (f32 elems) = 256 B
TABW = NPC + 1                   # per-owner window rows (incl. zero row)
GMAX = 96                        # max columns per dma_gather tile

LAST_RESULTS = None
LAST_NC = None


# --------------------------------------------------------------------------
# host-side index preprocessing (indices only, no feature payloads)
# --------------------------------------------------------------------------
def _prep_indices(src, dst):
    """Per (core, phase) degree-sorted padded layouts, common D across cores.

    Returns dict with:
      D       [8, NCH]  per-phase common padded chunk degrees
      off     [8, NCH]  column offsets within each phase's stream
      sumD    [8]       columns per phase
      groups  [8][...]  per phase: list of (k0, nk, colstart, dg) gather tiles
      Rvalid  [8]       valid scatter tokens per phase (common)
      gidx    [ncores][128, GL16]  wrapped int16 gather streams
      sidx    [ncores][128, 8*NPAD//16]  wrapped int16 scatter streams
      deginv  [ncores][128, NCH] f32
      phase_base16 [8]  column offset (in int16 cols) of each phase in gidx
    """
    E = src.shape[0]
    core = dst // NPC
    o = (src // NPC).astype(np.int64)
    srcloc = (src - o * NPC).astype(np.int64)
    dstloc = (dst - core * NPC).astype(np.int64)

    # per (core, phase, node) degree
    deg_cpn = np.zeros((NCORES, NCORES, NPC), np.int64)
    flat = (core * NCORES + o) * NPC + dstloc
    cnt = np.bincount(flat, minlength=NCORES * NCORES * NPC)
    deg_cpn = cnt.reshape(NCORES, NCORES, NPC)

    # total degree per (core, node) for deginv
    deg_tot = deg_cpn.sum(axis=1)  # [core, node]

    # per (core, phase): rank nodes by deg desc
    order = np.argsort(-deg_cpn, axis=2, kind="stable")  # [c, o, NPC]
    dsort = np.take_along_axis(deg_cpn, order, axis=2)   # sorted degrees
    # chunk max = first element of each chunk (desc order); pad to NPAD
    dpad = np.zeros((NCORES, NCORES, NPAD), np.int64)
    dpad[:, :, :NPC] = dsort
    Dc = dpad.reshape(NCORES, NCORES, NCH, P)[:, :, :, 0]  # [c, o, NCH]
    D = Dc.max(axis=0)                                     # [o, NCH] common
    off = np.zeros((NCORES, NCH), np.int64)
    off[:, 1:] = np.cumsum(D, axis=1)[:, :-1]
    sumD = D.sum(axis=1)                                   # [o]
    Rvalid = 128 * (D > 0).sum(axis=1)                     # [o]

    # rank of each node per (c, o)
    rank = np.empty((NCORES, NCORES, NPC), np.int64)
    np.put_along_axis(rank, order,
                      np.broadcast_to(np.arange(NPC), (NCORES, NCORES, NPC)),
                      axis=2)

    # gather tiles per phase (common)
    groups = []
    for oo in range(NCORES):
        g = []
        k0 = 0
        while k0 < NCH:
            dg = 0
            nk = 0
            while k0 + nk < NCH:
                dk = int(D[oo, k0 + nk])
                if nk > 0 and dg + dk > GMAX:
                    break
                dg += dk
                nk += 1
                if dg >= GMAX:
                    break
            g.append((k0, nk, int(off[oo, k0]), dg))
            k0 += nk
        # drop trailing zero-degree tiles
        g = [t for t in g if t[3] > 0]
        groups.append(g)

    GL = int(sumD.sum()) * P                 # total gather stream length
    phase_baseL = np.zeros(NCORES + 1, np.int64)
    phase_baseL[1:] = np.cumsum(sumD * P)

    # build per-core streams
    gidx_l, sidx_l, deginv_l = [], [], []
    # edge sort: by (core, phase, node, j)
    key = ((core * NCORES + o) * NPC + dstloc)
    eorder = np.argsort(key, kind="stable")
    key_s = key[eorder]
    srcloc_s = srcloc[eorder]
    starts = np.zeros(NCORES * NCORES * NPC + 1, np.int64)
    starts[1:] = np.cumsum(cnt)
    j_s = np.arange(E, dtype=np.int64) - starts[key_s]
    core_s = key_s // (NCORES * NPC)
    o_s = (key_s // NPC) % NCORES
    n_s = key_s % NPC

    for c in range(NCORES):
        gstream = np.full(GL, NPC, np.int16)   # dummy -> zero row
        sel = core_s == c
        oo = o_s[sel]
        nn = n_s[sel]
        jj = j_s[sel]
        rr = rank[c, oo, nn]
        kk = rr // P
        pp = rr % P
        pos = phase_baseL[oo] + (off[oo, kk] + jj) * P + pp
        gstream[pos] = srcloc_s[sel].astype(np.int16)
        gw = np.tile(gstream.reshape(GL // 16, 16).T, (8, 1))
        gidx_l.append(np.ascontiguousarray(gw))

        sstream = np.full(NCORES * NPAD, -1, np.int16)
        for oo2 in range(NCORES):
            R = int(Rvalid[oo2])
            perm = order[c, oo2]  # rank r -> node
            s = np.full(NPAD, -1, np.int16)
            s[:min(R, NPC)] = perm[:min(R, NPC)].astype(np.int16)
            if R > NPC:
                # pad ranks inside the valid prefix: route their (all-zero)
                # partial rows to distinct spare accumulator rows
                s[NPC:R] = np.arange(NPC, R, dtype=np.int16)
            sstream[oo2 * NPAD: (oo2 + 1) * NPAD] = s
        sw = np.tile(sstream.reshape(NCORES * NPAD // 16, 16).T, (8, 1))
        sidx_l.append(np.ascontiguousarray(sw))

        dinv = np.where(deg_tot[c] > 0, 1.0 / np.maximum(deg_tot[c], 1), 0.0)
        dpadv = np.zeros(NPAD, np.float32)
        dpadv[:NPC] = dinv
        deginv_l.append(np.ascontiguousarray(
            dpadv.reshape(NCH, P).T.astype(np.float32)))  # [p, k]

    return dict(D=D, off=off, sumD=sumD, groups=groups, Rvalid=Rvalid,
                gidx=gidx_l, sidx=sidx_l, deginv=deginv_l,
                phase_baseL=phase_baseL, GL=GL)


# --------------------------------------------------------------------------
# device program (identical for all cores)
# --------------------------------------------------------------------------
def _build_program(meta, f_in, f_hid, f_out):
    fh = f_hid + 1
    D = meta["D"]
    off = meta["off"]
    groups = meta["groups"]
    Rvalid = meta["Rvalid"]
    phase_baseL = meta["phase_baseL"]
    GL = meta["GL"]
    NTAB = NCORES * TABW

    nc = bacc.Bacc("TRN2", target_bir_lowering=False, debug=False,
                   num_devices=NCORES)

    feat_pad = nc.dram_tensor("feat_pad", [NTAB, STEP], F32,
                              kind="ExternalInput")
    featT_c = nc.dram_tensor("featT_c", [f_in, NPAD], F32,
                             kind="ExternalInput")
    gidx_d = nc.dram_tensor("gidx", [128, GL // 16], I16,
                            kind="ExternalInput")
    sidx_d = nc.dram_tensor("sidx", [128, NCORES * NPAD // 16], I16,
                            kind="ExternalInput")
    deginv_d = nc.dram_tensor("deginv", [128, NCH], F32,
                              kind="ExternalInput")
    w1s_d = nc.dram_tensor("w1s", [f_in, fh], F32, kind="ExternalInput")
    w1n_d = nc.dram_tensor("w1n", [f_in, fh], F32, kind="ExternalInput")
    b1_d = nc.dram_tensor("b1a", [fh, 1], F32, kind="ExternalInput")
    w2s_d = nc.dram_tensor("w2s", [fh, f_out], F32, kind="ExternalInput")
    w2n_d = nc.dram_tensor("w2n", [fh, f_out], F32, kind="ExternalInput")

    out_d = nc.dram_tensor("out_blk", [NPAD, f_out], F32,
                           kind="ExternalOutput")

    acc1 = nc.dram_tensor("acc1", [NPAD, STEP], F32)
    acc2 = nc.dram_tensor("acc2", [NPAD, STEP], F32)
    p_blk = nc.dram_tensor("p_blk", [NPAD, f_out], F32)
    p_cat = nc.dram_tensor("p_cat", [NCORES * NPAD, f_out], F32,
                           addr_space="Shared")
    p_pad = nc.dram_tensor("p_pad", [NTAB, STEP], F32)

    with tile.TileContext(nc) as tc:
        with (
            tc.tile_pool(name="const", bufs=1) as cpool,
            tc.tile_pool(name="persist", bufs=1) as ppool,
            tc.tile_pool(name="gather", bufs=2) as gpool,
            tc.tile_pool(name="gidx", bufs=2) as gipool,
            tc.tile_pool(name="sidxp", bufs=2) as sipool,
            tc.tile_pool(name="partial", bufs=1) as rpool,
            tc.tile_pool(name="ft", bufs=2) as fpool,
            tc.tile_pool(name="work", bufs=3) as wpool,
            tc.tile_pool(name="psA", bufs=2, space="PSUM") as psA,
            tc.tile_pool(name="psB", bufs=2, space="PSUM") as psB,
        ):
            # ---- constants
            ident = cpool.tile([P, P], F32, tag="ident")
            make_identity(nc, ident[:])
            w1s = cpool.tile([f_in, fh], F32, tag="w1s")
            nc.sync.dma_start(out=w1s[:], in_=w1s_d[:])
            w1n = cpool.tile([f_in, fh], F32, tag="w1n")
            nc.sync.dma_start(out=w1n[:], in_=w1n_d[:])
            b1 = cpool.tile([fh, 1], F32, tag="b1")
            nc.sync.dma_start(out=b1[:], in_=b1_d[:])
            w2s = cpool.tile([fh, f_out], F32, tag="w2s")
            nc.sync.dma_start(out=w2s[:], in_=w2s_d[:])
            w2n = cpool.tile([fh, f_out], F32, tag="w2n")
            nc.sync.dma_start(out=w2n[:], in_=w2n_d[:])
            w2s_b = cpool.tile([fh, f_out], BF16, tag="w2s_b")
            nc.vector.tensor_copy(out=w2s_b[:], in_=w2s[:])
            w2n_b = cpool.tile([fh, f_out], BF16, tag="w2n_b")
            nc.vector.tensor_copy(out=w2n_b[:], in_=w2n[:])
            deginv = cpool.tile([128, NCH], F32, tag="deginv")
            nc.sync.dma_start(out=deginv[:], in_=deginv_d[:])
            zeros = cpool.tile([128, 3136], F32, tag="zeros")
            nc.vector.memset(zeros[:], 0.0)

            # zero accumulators: acc[a*128+p, e] via [p, a, e] view
            for accd in (acc1, acc2):
                for h in range(4):
                    nc.sync.dma_start(
                        out=accd[:].rearrange("(a p) e -> p a e", p=P)[
                            :, h * 49:(h + 1) * 49, :],
                        in_=zeros[:, :49 * STEP].rearrange(
                            "p (a e) -> p a e", e=STEP))

            # persistent hT (bf16) for both layers
            hT = ppool.tile([fh, NPAD], BF16, tag="hT")

            # ---------------- layer pass helper ----------------
            def layer_pass(table, accd, fdim):
                """8 phase gathers + reduces + scatter-adds into accd."""
                for o in range(NCORES):
                    ptile = rpool.tile([P, NCH * f_in], F32, tag="pt")
                    for (k0, nk, colstart, dg) in groups[o]:
                        it = gipool.tile([128, GMAX * 8], I16, tag="gi")
                        c16 = (int(phase_baseL[o]) + colstart * P) // 16
                        nc.sync.dma_start(
                            out=it[:, : dg * 8],
                            in_=gidx_d[:, c16: c16 + dg * 8])
                        gt = gpool.tile([P, GMAX * STEP], F32, tag="gt")
                        nc.gpsimd.dma_gather(
                            out_ap=gt[:, : dg * STEP].rearrange(
                                "p (j e) -> p j e", e=STEP),
                            in_ap=table[o * TABW:(o + 1) * TABW, :],
                            idxs_ap=it[:, : dg * 8],
                            num_idxs=dg * P,
                            num_idxs_reg=dg * P,
                            elem_size=STEP,
                            single_packet=False,
                        )
                        # batched segmented reduces over equal-D chunk runs
                        kk = 0
                        while kk < nk:
                            k = k0 + kk
                            dk = int(D[o, k])
                            if dk == 0:
                                kk += 1
                                continue
                            m = 1
                            while (kk + m < nk
                                   and int(D[o, k0 + kk + m]) == dk):
                                m += 1
                            o64 = (int(off[o, k]) - colstart) * STEP
                            src_v = gt[:, o64: o64 + m * dk * STEP].rearrange(
                                "p (m j e) -> p m e j", m=m, e=STEP
                            )[:, :, :fdim, :]
                            nc.vector.tensor_reduce(
                                out=ptile[:, :NCH * fdim].rearrange(
                                    "p (k e) -> p k e", e=fdim
                                )[:, k: k + m, :],
                                in_=src_v,
                                axis=mybir.AxisListType.X,
                                op=mybir.AluOpType.add)
                            kk += m
                    # scatter-add this phase's partial into accd
                    # (split into 4 sub-scatters of 49 chunks: HW caps the
                    # per-instruction token count around 8k)
                    st = sipool.tile([128, NPAD // 16], I16, tag="si")
                    nc.sync.dma_start(
                        out=st[:],
                        in_=sidx_d[:, o * (NPAD // 16):(o + 1) * (NPAD // 16)])
                    R = int(Rvalid[o])
                    for h in range(4):
                        t0 = h * 49 * P
                        nv = min(max(R - t0, 0), 49 * P)
                        if nv == 0:
                            break
                        nc.gpsimd.dma_scatter_add(
                            out_ap=accd[:, :fdim],
                            in_ap=ptile[:, h * 49 * fdim:(h + 1) * 49 * fdim
                                        ].rearrange("p (k e) -> p k e",
                                                    e=fdim),
                            idxs_ap=st[:, h * 49 * 8:(h + 1) * 49 * 8],
                            num_idxs=49 * P,
                            num_idxs_reg=nv,
                            elem_size=fdim,
                            elem_step=STEP,
                            single_packet=False,
                        )

            # ---------------- LAYER 1 ----------------
            layer_pass(feat_pad, acc1, f_in)

            # readback acc1 -> agg [p, k*f_in]
            agg = ppool.tile([P, NCH * f_in], F32, tag="agg")
            nc.sync.dma_start(
                out=agg[:].rearrange("p (k e) -> p k e", e=f_in),
                in_=acc1[:, :f_in].rearrange("(k p) e -> p k e", p=P))

            # per chunk: mean, transpose, matmuls, relu -> hT ; p -> p_blk
            for k in range(NCH):
                if k % 4 == 0:
                    ft = fpool.tile([f_in, 512], F32, tag="ft")
                    w = min(512, NPAD - k * P)
                    nc.sync.dma_start(
                        out=ft[:, :w], in_=featT_c[:, k * P: k * P + w])
                mean = wpool.tile([P, f_in], F32, tag="mean")
                nc.vector.tensor_scalar(
                    out=mean[:], in0=agg[:, k * f_in:(k + 1) * f_in],
                    scalar1=deginv[:, k:k + 1], scalar2=None,
                    op0=mybir.AluOpType.mult)
                mT_ps = psB.tile([f_in, P], F32, tag="mT_ps")
                nc.tensor.transpose(out=mT_ps[:], in_=mean[:],
                                    identity=ident[:])
                mT = wpool.tile([f_in, P], F32, tag="mT")
                nc.vector.tensor_copy(out=mT[:], in_=mT_ps[:])
                hps = psA.tile([fh, P], F32, tag="hps")
                nc.tensor.matmul(out=hps[:], lhsT=w1s[:],
                                 rhs=ft[:, (k % 4) * P:(k % 4 + 1) * P],
                                 start=True, stop=False)
                nc.tensor.matmul(out=hps[:], lhsT=w1n[:], rhs=mT[:],
                                 start=False, stop=True)
                nc.scalar.activation(
                    out=hT[:, k * P:(k + 1) * P], in_=hps[:],
                    func=mybir.ActivationFunctionType.Relu, bias=b1[:, :1])
                pps = psB.tile([P, f_out], F32, tag="pps")
                nc.tensor.matmul(out=pps[:], lhsT=hT[:, k * P:(k + 1) * P],
                                 rhs=w2n_b[:], start=True, stop=True)
                p_sb = wpool.tile([P, f_out], F32, tag="p_sb")
                nc.vector.tensor_copy(out=p_sb[:], in_=pps[:])
                nc.sync.dma_start(out=p_blk[k * P:(k + 1) * P, :], in_=p_sb[:])

            # ---------------- exchange p ----------------
            nc.gpsimd.collective_compute(
                "AllGather",
                mybir.AluOpType.bypass,
                replica_groups=[list(range(NCORES))],
                ins=[p_blk[:]],
                outs=[p_cat[:]],
            )
            # restride into p_pad (+ zero dummy rows)
            for oo in range(NCORES):
                nc.sync.dma_start(
                    out=p_pad[oo * TABW: oo * TABW + NPC, :f_out],
                    in_=p_cat[oo * NPAD: oo * NPAD + NPC, :])
            nc.sync.dma_start(
                out=p_pad[:].rearrange("(o r) e -> o r e", o=NCORES)[
                    :, NPC, :f_out],
                in_=zeros[:NCORES, :f_out])

            # ---------------- LAYER 2 ----------------
            layer_pass(p_pad, acc2, f_out)

            agg2 = ppool.tile([P, NCH * f_in], F32, tag="agg")
            nc.sync.dma_start(
                out=agg2[:, :NCH * f_out].rearrange(
                    "p (k e) -> p k e", e=f_out),
                in_=acc2[:, :f_out].rearrange("(k p) e -> p k e", p=P))

            tall = ppool.tile([P, NCH * f_out], F32, tag="tall")
            for k in range(NCH):
                sps = psA.tile([P, f_out], F32, tag="sps")
                nc.tensor.matmul(out=sps[:], lhsT=hT[:, k * P:(k + 1) * P],
                                 rhs=w2s_b[:], start=True, stop=True)
                mean2 = wpool.tile([P, f_out], F32, tag="mean2")
                nc.vector.tensor_scalar(
                    out=mean2[:], in0=agg2[:, k * f_out:(k + 1) * f_out],
                    scalar1=deginv[:, k:k + 1], scalar2=None,
                    op0=mybir.AluOpType.mult)
                nc.vector.tensor_tensor(
                    out=tall[:, k * f_out:(k + 1) * f_out],
                    in0=sps[:], in1=mean2[:], op=mybir.AluOpType.add)

            # ---------------- batched log-softmax ----------------
            ex = rpool.tile([P, NCH * f_in], F32, tag="pt")  # reuse pool
            nc.scalar.activation(out=ex[:, :NCH * f_out], in_=tall[:],
                                 func=mybir.ActivationFunctionType.Exp)
            se = cpool.tile([P, NCH], F32, tag="se")
            nc.vector.tensor_reduce(
                out=se[:],
                in_=ex[:, :NCH * f_out].rearrange("p (k e) -> p k e", e=f_out),
                axis=mybir.AxisListType.X, op=mybir.AluOpType.add)
            ln = cpool.tile([P, NCH], F32, tag="ln")
            nc.scalar.activation(out=ln[:], in_=se[:],
                                 func=mybir.ActivationFunctionType.Ln)
            for k in range(NCH):
                o_sb = wpool.tile([P, f_out], F32, tag="o_sb")
                nc.vector.tensor_scalar(
                    out=o_sb[:], in0=tall[:, k * f_out:(k + 1) * f_out],
                    scalar1=ln[:, k:k + 1], scalar2=None,
                    op0=mybir.AluOpType.subtract)
                nc.sync.dma_start(out=out_d[k * P:(k + 1) * P, :], in_=o_sb[:])

    return nc


# --------------------------------------------------------------------------
# public entry
# --------------------------------------------------------------------------
def _run(feat, src, dst, W1_self, W1_neigh, b1, W2_self, W2_neigh, b2,
         trace=False):
    global LAST_RESULTS, LAST_NC
    n_nodes, f_in = feat.shape
    f_hid = W1_self.shape[1]
    f_out = W2_self.shape[1]
    fh = f_hid + 1

    src = np.asarray(src).astype(np.int64, copy=False)
    dst = np.asarray(dst).astype(np.int64, copy=False)
    feat = np.asarray(feat, dtype=np.float32)

    meta = _prep_indices(src, dst)

    nc = _build_program(meta, f_in, f_hid, f_out)
    nc.compile()
    LAST_NC = nc

    # host-side tensors (formatting only)
    feat_pad = np.zeros((NCORES * TABW, STEP), np.float32)
    for o in range(NCORES):
        feat_pad[o * TABW: o * TABW + NPC, :f_in] = \
            feat[o * NPC:(o + 1) * NPC]

    w1s_aug = np.zeros((f_in, fh), np.float32)
    w1s_aug[:, :f_hid] = W1_self
    w1n_aug = np.zeros((f_in, fh), np.float32)
    w1n_aug[:, :f_hid] = W1_neigh
    b1_aug = np.zeros((fh, 1), np.float32)
    b1_aug[:f_hid, 0] = b1
    b1_aug[f_hid, 0] = 1.0
    w2s_aug = np.zeros((fh, f_out), np.float32)
    w2s_aug[:f_hid] = W2_self
    w2s_aug[f_hid] = b2
    w2n_aug = np.zeros((fh, f_out), np.float32)
    w2n_aug[:f_hid] = W2_neigh

    in_maps = []
    for c in range(NCORES):
        fT = np.zeros((f_in, NPAD), np.float32)
        fT[:, :NPC] = feat[c * NPC:(c + 1) * NPC].T
        in_maps.append({
            "feat_pad": feat_pad,
            "featT_c": np.ascontiguousarray(fT),
            "gidx": meta["gidx"][c],
            "sidx": meta["sidx"][c],
            "deginv": meta["deginv"][c],
            "w1s": w1s_aug,
            "w1n": w1n_aug,
            "b1a": b1_aug,
            "w2s": w2s_aug,
            "w2n": w2n_aug,
        })

    res = run_bass_kernel_spmd(nc, in_maps, list(range(NCORES)), trace=trace)
    LAST_RESULTS = res

    out = np.empty((n_nodes, f_out), np.float32)
    for c in range(NCORES):
        out[c * NPC:(c + 1) * NPC] = res.results[c]["out_blk"][:NPC]
    return out


def kernel(feat, src, dst, W1_self, W1_neigh, b1, W2_self, W2_neigh, b2):
    return _run(
        np.asarray(feat), np.asarray(src), np.asarray(dst),
        np.asarray(W1_self, dtype=np.float32),
        np.asarray(W1_neigh, dtype=np.float32),
        np.asarray(b1, dtype=np.float32),
        np.asarray(W2_self, dtype=np.float32),
        np.asarray(W2_neigh, dtype=np.float32),
        np.asarray(b2, dtype=np.float32),
        trace=bool(int(os.environ.get("KERNEL_TRACE", "0"))),
    )
